# revision 1
# baseline (speedup 1.0000x reference)
"""NeuralMMU Trainium2 kernel.

Pipeline per core (131072 addrs, 64 iterations x 2048 addrs):
  1. SP-triggered DMA of host-unpacked bit planes -> SBUF [96, 8192] u8
     (4 iters per DMA); partition q = 32s + k holds bit k (replicated 3x,
     s = 0..2), col j*2048 + 512g + c -> addr of iter j, block g
  2. DVE tensor_copy u8 -> bf16 bits [96, 2048] per iter
  3. 4x bf16 matmul k=96: bits @ (W1hi; W1mid; W1lo) -> PSUM [128,2048]
     (exact 3-way bf16 split of f32 W1, summed in the contraction dim)
  4. ACT Gelu(+b1): PSUM -> SBUF h [128,2048]
  5. 4x f32 matmul (PE col tiles 32g): h @ W2ext -> PSUM [128,512]
  6. DVE is_gt per-partition threshold (0.5 - b2): -> bf16 bits
  7. ONE bf16 matmul, block-diagonal [128,8] weights: packs all 4
     col-bands' 26 bits as lo13/hi13 in a single 512-row pass -> PSUM
  8. DVE copy PSUM -> SBUF accumulator [8,4096] (8 iters)
  9. 1x SP-triggered DMA [8,4096] per 8 iters -> DRAM;
     host combines lo + 8192*hi -> int64

The loop is software-pipelined two-deep so the PE never stalls:
PE order per iter t is L1(t+1), L2(t), pack(t-1); DVE converts bits
for t+2 while ACT runs Gelu(t) and PE runs L2(t).  This hides both
the L1(t)->Gelu(t)->L2(t) chain (Gelu finishes ~2.4us before L2
needs it) and the L2(t)->threshold(t)->pack(t) chain (threshold has
a full iteration of slack).  A small iter-0-only input DMA (R0t)
hides most of the first group-DMA latency at startup.

PE busy is ~96% of total; the f32 L2 (4 cyc/row) is optimal for the
required exactness: logit threshold gaps go down to 2.5e-8, so the
contraction must be f32-exact, and an explicit 5-pair bf16 split
would move 10240 rows/iter vs f32's effective 8192.

HW-validated: ~299 us/core, 1/1048576 mismatch (the one addr with a
2.5e-8 logit-threshold gap; same flip as a pure-f32 kernel).
"""

import numpy as np
from contextlib import ExitStack

import concourse.bass as bass
import concourse.mybir as mybir
import concourse.tile as tile
from concourse import bacc, bass_utils

B = 1_048_576
NCORES = 8
PER = B // NCORES          # 131072 addrs per core
BLK = 512                  # addrs per PE block
NBLK = 4                   # blocks per iteration
CHUNK = NBLK * BLK         # 2048 addrs per iteration
N_ITERS = PER // CHUNK     # 64
GIN = 4                    # iters per input DMA
GOUT = 8                   # iters per output DMA set

F32 = mybir.dt.float32
BF16 = mybir.dt.bfloat16
U8 = mybir.dt.uint8
AF = mybir.ActivationFunctionType
ALU = mybir.AluOpType


def build_nc(n_iters: int = N_ITERS, act=AF.Gelu) -> bass.Bass:
    nc = bacc.Bacc("TRN2")
    assert n_iters % GOUT == 0 and n_iters % GIN == 0

    bp = nc.dram_tensor("bp", [n_iters // GIN, 96, GIN * CHUNK], U8,
                        kind="ExternalInput")
    cst_d = nc.dram_tensor("cst", [128, 102], F32, kind="ExternalInput")
    outp = nc.dram_tensor("outp", [2 * NBLK, n_iters // GOUT, GOUT * BLK], F32,
                          kind="ExternalOutput")

    with ExitStack() as ctx:
        tc = ctx.enter_context(tile.TileContext(nc))
        const = ctx.enter_context(tc.tile_pool(name="const", bufs=1))
        rpool = ctx.enter_context(tc.tile_pool(name="rp", bufs=2))
        bitsp = ctx.enter_context(tc.tile_pool(name="bitsp", bufs=2))
        hp = ctx.enter_context(tc.tile_pool(name="hp", bufs=2))
        bop = ctx.enter_context(tc.tile_pool(name="bop", bufs=2))
        pksp = ctx.enter_context(tc.tile_pool(name="pksp", bufs=2))
        hprep = ctx.enter_context(tc.tile_pool(name="hprep", bufs=1, space="PSUM"))
        l2p = ctx.enter_context(tc.tile_pool(name="l2p", bufs=2, space="PSUM"))
        pkp = ctx.enter_context(tc.tile_pool(name="pkp", bufs=2, space="PSUM"))

        cst = const.tile([128, 102], F32)
        nc.sync.dma_start(cst[:], cst_d[:])
        w1b = cst[:, 0:64].bitcast(BF16)     # [128, 128] bf16; rows 0-95 used
        w2s = cst[:, 64:96]
        b1c = cst[:, 96:97]
        thc = cst[:, 97:98]
        pwc = cst[:, 98:102].bitcast(BF16)   # [128, 8] block-diag pack weights

        R = None
        pks = None

        def load_input(t):
            nonlocal R
            if t % GIN == 0:
                R = rpool.tile([96, GIN * CHUNK], U8)
                nc.sync.dma_start(R[:], bp[t // GIN])

        def convert(t):
            bits = bitsp.tile([96, CHUNK], BF16)
            nc.vector.tensor_copy(
                bits[:], R[:, CHUNK * (t % GIN):CHUNK * (t % GIN + 1)]
            )
            return bits

        def l1mm(bits):
            hpre = hprep.tile([128, CHUNK], F32)
            for g in range(NBLK):
                nc.tensor.matmul(
                    hpre[:, BLK * g:BLK * (g + 1)],
                    w1b[0:96, :],
                    bits[0:96, BLK * g:BLK * (g + 1)],
                    start=True, stop=True, tile_position=(0, 0),
                )
            return hpre

        R0t = rpool.tile([96, CHUNK], U8)
        nc.sync.dma_start(R0t[:], bp[0, :, 0:CHUNK])
        load_input(0)
        bits0 = bitsp.tile([96, CHUNK], BF16)
        nc.vector.tensor_copy(bits0[:], R0t[:])
        hpre = l1mm(bits0)
        if n_iters > 1:
            bits_next = convert(1)

        bo_prev = None

        def pack_and_store(tp):
            nonlocal pks
            pk = pkp.tile([2 * NBLK, BLK], F32)
            nc.tensor.matmul(
                pk[:],
                pwc[:],
                bo_prev[:],
                start=True, stop=True, tile_position=(0, 0),
            )
            if tp % GOUT == 0:
                pks = pksp.tile([2 * NBLK, GOUT * BLK], F32)
            nc.vector.tensor_copy(
                pks[:, BLK * (tp % GOUT):BLK * (tp % GOUT + 1)], pk[:]
            )
            if tp % GOUT == GOUT - 1:
                nc.sync.dma_start(outp[:, tp // GOUT, :], pks[:])

        for t in range(n_iters):
            h = hp.tile([128, CHUNK], F32)
            nc.scalar.activation(h[:], hpre[:], act, bias=b1c, scale=1.0)

            if t + 2 < n_iters:
                load_input(t + 2)
                bits_fut = convert(t + 2)

            if t + 1 < n_iters:
                hpre = l1mm(bits_next)
                if t + 2 < n_iters:
                    bits_next = bits_fut

            l2o = l2p.tile([128, BLK], F32)
            for g in range(NBLK):
                nc.tensor.matmul(
                    l2o[32 * g:32 * (g + 1), :],
                    w2s[:],
                    h[:, BLK * g:BLK * (g + 1)],
                    start=True, stop=True, tile_position=(0, 32 * g),
                )

            if t > 0:
                pack_and_store(t - 1)

            bo = bop.tile([128, BLK], BF16)
            nc.vector.tensor_scalar(
                bo[:], l2o[:], thc, None, op0=ALU.is_gt,
            )
            bo_prev = bo

        pack_and_store(n_iters - 1)

    return nc


def make_const_inputs(W1, b1, W2, b2):
    import ml_dtypes

    w1 = np.ascontiguousarray(W1[0:32, :], dtype=np.float32)
    hi = w1.astype(ml_dtypes.bfloat16)
    mid = (w1 - hi.astype(np.float32)).astype(ml_dtypes.bfloat16)
    lo = (w1 - hi.astype(np.float32) - mid.astype(np.float32)).astype(
        ml_dtypes.bfloat16
    )
    w1b = np.zeros((128, 128), dtype=ml_dtypes.bfloat16)
    w1b[0:32] = hi
    w1b[32:64] = mid
    w1b[64:96] = lo

    w2s = np.zeros((128, 32), dtype=np.float32)
    w2s[:, :26] = W2[:, :26]
    b1c = np.asarray(b1, dtype=np.float32).reshape(128, 1)
    thc = np.full((128, 1), 1e30, dtype=np.float32)
    pwc = np.zeros((128, 8), dtype=np.float32)
    for g in range(4):
        thc[32 * g:32 * g + 26, 0] = 0.5 - np.asarray(b2[:26], dtype=np.float32)
        for i in range(13):
            pwc[32 * g + i, 2 * g] = float(1 << i)
            pwc[32 * g + 13 + i, 2 * g + 1] = float(1 << i)
    cst = np.empty((128, 102), dtype=np.float32)
    cst[:, 0:64] = np.ascontiguousarray(w1b).view(np.float32)
    cst[:, 64:96] = w2s
    cst[:, 96:97] = b1c
    cst[:, 97:98] = thc
    cst[:, 98:102] = (
        np.ascontiguousarray(pwc.astype(ml_dtypes.bfloat16)).view(np.float32)
    )
    return {"cst": cst}


def make_bit_planes(virtual_addr, n_iters: int = N_ITERS):
    """Per-core [n_iters//GIN, 96, GIN*2048] u8 0/1 bit-plane arrays.

    Partition 32s + k (s = 0..2 replication), col j*2048 + 512g + c =
    bit k of addr (GIN*tt + j)*2048 + g*512 + c.
    """
    va32 = np.asarray(virtual_addr).astype(np.uint32)
    per = n_iters * CHUNK
    ncores = va32.size // per
    out = []
    for c in range(ncores):
        seg = va32[c * per:(c + 1) * per]
        byt = seg.view(np.uint8).reshape(n_iters // GIN, GIN, NBLK, BLK, 4)
        bits = np.unpackbits(byt, axis=-1, bitorder="little")
        # (tt, j, g, c, k) -> (tt, k, j, g, c)
        pl = bits.transpose(0, 4, 1, 2, 3).reshape(n_iters // GIN, 32, GIN * CHUNK)
        out.append(np.ascontiguousarray(np.concatenate([pl, pl, pl], axis=1)))
    return out


def combine_output(o, n_iters: int = N_ITERS):
    """[8, n_iters//GOUT, GOUT*512] f32 -> [per] int64."""
    arr = o.reshape(NBLK, 2, n_iters // GOUT, GOUT, BLK)
    lo = arr[:, 0].transpose(1, 2, 0, 3).reshape(-1).astype(np.int64)
    hi = arr[:, 1].transpose(1, 2, 0, 3).reshape(-1).astype(np.int64)
    return lo + 8192 * hi


_NC_CACHE = {}
TRACE = False
LAST_RES = None


def kernel(virtual_addr, W1, b1, W2, b2):
    global LAST_RES
    if "nc" not in _NC_CACHE:
        nc = build_nc(N_ITERS)
        nc.finalize()
        _NC_CACHE["nc"] = nc
    nc = _NC_CACHE["nc"]

    consts = make_const_inputs(W1, b1, W2, b2)
    planes = make_bit_planes(virtual_addr, N_ITERS)
    in_maps = [{"bp": planes[c], **consts} for c in range(NCORES)]

    res = bass_utils.run_bass_kernel_spmd(
        nc, in_maps, list(range(NCORES)), trace=TRACE
    )
    LAST_RES = res

    outs = [combine_output(res.results[c]["outp"]) for c in range(NCORES)]
    return np.concatenate(outs)



# revision 12
# speedup vs baseline: 1.6961x; 1.6961x over previous
"""NeuralMMU Trainium2 kernel (v2: ACT-bound pipeline).

Per core: 131072 addrs, 64 iterations x 2048 addrs.

Engine plan per iteration t (steady state, ~1.9us period):
  ACT   Gelu(+b1): hpre slot(t) PSUM [128,2048] -> h(t) SBUF f32.
        One op per iter; this is the bottleneck engine (~1892 ns).
  PE    L2(t-1): 16 matmuls with SWAPPED operands: stationary lhsT =
        h(t-1)[:, 128c:128c+128] (f32, exact), moving rhs = W2 [128,26]
        f32 -> batch-major logits [128 batch, 26] written into the TAIL
        416 f32 of psum slot(t-1) (bank 3), which gelu(t-1) has already
        consumed.  26 cols * 4 cyc/row * 16 = 1664 cyc.
        L1(t+1): 4 bf16 matmuls k=96 (3-way bf16 split of W1, exact to
        ~2^-27) from host-prepared bf16 bit planes -> slot(t+1).
        Blocks g=0..2 issue early; block g=3 (tail bank) waits until the
        DVE threshold has read slot(t+1)'s previous logits.
  DVE   TT is_gt vs per-logit threshold vector (0.5 - b2[j], f32,
        partition-broadcast) -> bits bf16; TT mult by 2^(j%13) weight
        vector; tensor_reduce sum [128,16,2,13] -> packed lo/hi
        [128,32] f32 into an 8-iter accumulator.
  DMA   in: [96,4096] bf16 planes per 2 iters; out: [128,256] f32 per
        8 iters.  Host packs bit planes and combines lo+8192*hi.

PSUM: exactly 8 banks = 2 slots x [128,2048] f32; L2 output aliases the
tail of the slot (time-multiplexed with hpre data).

Numerics are f32-exact end-to-end except the 3-way-bf16 W1 split
(~2^-27) and the ACT Gelu LUT, identical to the f32 baseline (1/1M
mismatch there).
"""

import numpy as np
from contextlib import ExitStack

import concourse.bass as bass
import concourse.mybir as mybir
import concourse.tile as tile
from concourse import bacc, bass_utils

B = 1_048_576
NCORES = 8
PER = B // NCORES          # 131072 addrs per core
BLK = 512                  # addrs per L1 PE block
NBLK = 4                   # L1 blocks per iteration
CH = 128                   # addrs per L2 chunk (stationary width)
NCH = 16                   # L2 chunks per iteration
CHUNK = NBLK * BLK         # 2048 addrs per iteration
N_ITERS = PER // CHUNK     # 64
GIN = 2                    # iters per input DMA
GOUT = 8                   # iters per output DMA
NLOG = 26                  # logits per addr
LW = NCH * NLOG            # 416 logit cols per iteration
L2C0 = CHUNK - LW          # 1632: tail offset of logits in the psum slot

F32 = mybir.dt.float32
BF16 = mybir.dt.bfloat16
AF = mybir.ActivationFunctionType
ALU = mybir.AluOpType

# cst columns (f32): w1b 0:64 (bf16x128), b1c 64:65, w2f 65:91,
# wvec 91:299 (bf16 x416), thvec 299:715
CW1, CB1, CW2, CWV, CTH, CTOT = 0, 64, 65, 91, 299, 715


def build_nc(n_iters: int = N_ITERS, act=AF.Gelu) -> bass.Bass:
    nc = bacc.Bacc("TRN2")
    assert n_iters % GOUT == 0 and n_iters % GIN == 0

    bp = nc.dram_tensor("bp", [n_iters // GIN, 96, GIN * CHUNK], BF16,
                        kind="ExternalInput")
    cst_d = nc.dram_tensor("cst", [128, CTOT], F32, kind="ExternalInput")
    outp = nc.dram_tensor("outp", [n_iters // GOUT, 128, GOUT * 32], F32,
                          kind="ExternalOutput")

    with ExitStack() as ctx:
        tc = ctx.enter_context(tile.TileContext(nc))
        const = ctx.enter_context(tc.tile_pool(name="const", bufs=1))
        rpool = ctx.enter_context(tc.tile_pool(name="rp", bufs=3))
        slotp = ctx.enter_context(
            tc.tile_pool(name="slotp", bufs=2, space="PSUM"))
        hp = ctx.enter_context(tc.tile_pool(name="hp", bufs=2))
        bop = ctx.enter_context(tc.tile_pool(name="bop", bufs=2))
        bwp = ctx.enter_context(tc.tile_pool(name="bwp", bufs=2))
        pksp = ctx.enter_context(tc.tile_pool(name="pksp", bufs=2))

        cst = const.tile([128, CTOT], F32)
        nc.sync.dma_start(cst[:], cst_d[:])
        w1b = cst[:, CW1:CB1].bitcast(BF16)      # [128,128] bf16; rows 0-95
        b1c = cst[:, CB1:CW2]
        w2f = cst[:, CW2:CWV]                    # [128, 26] f32
        wv = cst[:, CWV:CTH].bitcast(BF16)       # [128, 416] bf16
        thv = cst[:, CTH:CTOT]                   # [128, 416] f32

        R = {}
        slots = {}
        hs = {}
        bos = {}
        pks = None

        def load_input(g):
            if g < n_iters // GIN and g not in R:
                r = rpool.tile([96, GIN * CHUNK], BF16, name="r")
                nc.sync.dma_start(r[:], bp[g])
                R[g] = r

        def l1(t, lo_blocks):
            """lo_blocks=True: blocks 0..2; False: block 3 (tail bank)."""
            if t >= n_iters:
                return
            if lo_blocks:
                slots[t] = slotp.tile([128, CHUNK], F32, name="slot")
            r = R[t // GIN]
            s = slots[t]
            blocks = range(3) if lo_blocks else (3,)
            for g in blocks:
                c0 = CHUNK * (t % GIN) + BLK * g
                nc.tensor.matmul(
                    s[:, BLK * g:BLK * (g + 1)],
                    w1b[0:96, :],
                    r[0:96, c0:c0 + BLK],
                    start=True, stop=True, tile_position=(0, 0),
                )

        def gelu(t):
            h = hp.tile([128, CHUNK], F32, name="h")
            nc.scalar.activation(h[:], slots[t][:, 0:CHUNK], act,
                                 bias=b1c, scale=1.0)
            hs[t] = h

        def l2(t):
            h = hs.pop(t)
            s = slots[t]
            for c in range(NCH):
                nc.tensor.matmul(
                    s[:, L2C0 + NLOG * c:L2C0 + NLOG * (c + 1)],
                    h[:, CH * c:CH * (c + 1)],
                    w2f[:],
                    start=True, stop=True, tile_position=(0, 0),
                )

        def thresh(t):
            bo = bop.tile([128, LW], BF16, name="bo")
            nc.vector.tensor_tensor(bo[:], slots[t][:, L2C0:CHUNK], thv,
                                    op=ALU.is_gt)
            bos[t] = bo

        def pack(t):
            nonlocal pks
            bw = bwp.tile([128, LW], BF16, name="bw")
            nc.vector.tensor_tensor(bw[:], bos.pop(t)[:], wv, op=ALU.mult)
            if t % GOUT == 0:
                pks = pksp.tile([128, GOUT * 32], F32, name="pks")
            nc.vector.tensor_reduce(
                pks[:, 32 * (t % GOUT):32 * (t % GOUT + 1)],
                bw[:].rearrange("p (g x) -> p g x", x=13),
                axis=mybir.AxisListType.X,
                op=ALU.add,
            )
            if t % GOUT == GOUT - 1:
                nc.sync.dma_start(outp[t // GOUT], pks[:])

        # Prologue: planes for iters 0-3, L1(0).
        load_input(0)
        load_input(1)
        l1(0, True)
        l1(0, False)

        for t in range(n_iters):
            gelu(t)
            if t >= 1:
                l2(t - 1)
                thresh(t - 1)
            if t % GIN == 0:
                load_input(t // GIN + 2)
            l1(t + 1, True)
            if t >= 1:
                pack(t - 1)
            l1(t + 1, False)

        l2(n_iters - 1)
        thresh(n_iters - 1)
        pack(n_iters - 1)

    return nc


def make_const_inputs(W1, b1, W2, b2):
    import ml_dtypes

    w1 = np.ascontiguousarray(W1[0:32, :], dtype=np.float32)
    hi = w1.astype(ml_dtypes.bfloat16)
    mid = (w1 - hi.astype(np.float32)).astype(ml_dtypes.bfloat16)
    lo = (w1 - hi.astype(np.float32) - mid.astype(np.float32)).astype(
        ml_dtypes.bfloat16
    )
    w1b = np.zeros((128, 128), dtype=ml_dtypes.bfloat16)
    w1b[0:32] = hi
    w1b[32:64] = mid
    w1b[64:96] = lo

    cst = np.zeros((128, CTOT), dtype=np.float32)
    cst[:, CW1:CB1] = np.ascontiguousarray(w1b).view(np.float32)
    cst[:, CB1] = np.asarray(b1, dtype=np.float32)
    cst[:, CW2:CWV] = np.asarray(W2[:, :NLOG], dtype=np.float32)
    wvec = np.tile(
        np.concatenate([2.0 ** np.arange(13), 2.0 ** np.arange(13)]), NCH
    ).astype(ml_dtypes.bfloat16)        # [416]
    cst[:, CWV:CTH] = np.ascontiguousarray(wvec).view(np.float32)[None, :]
    thvec = np.tile(0.5 - np.asarray(b2[:NLOG], dtype=np.float32), NCH)
    cst[:, CTH:CTOT] = thvec[None, :]
    return {"cst": cst}


def make_bit_planes(virtual_addr, n_iters: int = N_ITERS):
    """Per-core [n_iters//GIN, 96, GIN*2048] bf16 0/1 bit planes.

    Partition 32s + k (s = 0..2 replication) of DMA group tt, col
    j*2048 + n = bit k of addr (GIN*tt + j)*2048 + n.
    """
    import ml_dtypes

    va32 = np.asarray(virtual_addr).astype(np.uint32)
    per = n_iters * CHUNK
    ncores = va32.size // per
    out = []
    for c in range(ncores):
        seg = va32[c * per:(c + 1) * per]
        byt = seg.view(np.uint8).reshape(n_iters // GIN, GIN * CHUNK, 4)
        bits = np.unpackbits(byt, axis=-1, bitorder="little")
        # (tt, n, k) -> (tt, k, n)
        pl = bits.transpose(0, 2, 1)
        pl3 = np.concatenate([pl, pl, pl], axis=1).astype(ml_dtypes.bfloat16)
        out.append(np.ascontiguousarray(pl3))
    return out


def combine_output(o, n_iters: int = N_ITERS):
    """[n_iters//GOUT, 128, GOUT*32] f32 -> [per] int64.

    col 32*ts + 2*c + half: lo/hi 13-bit halves of chunk c, iter
    GOUT*tt + ts; addr = CHUNK*t + CH*c + p.
    """
    arr = np.asarray(o, dtype=np.int64).reshape(
        n_iters // GOUT, 128, GOUT, NCH, 2)
    lo = arr[..., 0]                     # [tt, p, ts, c]
    hi = arr[..., 1]
    val = lo + 8192 * hi                 # [tt, p, ts, c]
    return val.transpose(0, 2, 3, 1).reshape(-1)


_NC_CACHE = {}
TRACE = False
LAST_RES = None


def kernel(virtual_addr, W1, b1, W2, b2):
    global LAST_RES
    if "nc" not in _NC_CACHE:
        nc = build_nc(N_ITERS)
        nc.finalize()
        _NC_CACHE["nc"] = nc
    nc = _NC_CACHE["nc"]

    consts = make_const_inputs(W1, b1, W2, b2)
    planes = make_bit_planes(virtual_addr, N_ITERS)
    in_maps = [{"bp": planes[c], **consts} for c in range(NCORES)]

    res = bass_utils.run_bass_kernel_spmd(
        nc, in_maps, list(range(NCORES)), trace=TRACE
    )
    LAST_RES = res

    outs = [combine_output(res.results[c]["outp"]) for c in range(NCORES)]
    return np.concatenate(outs)


# revision 15
# speedup vs baseline: 1.9122x; 1.1274x over previous
"""NeuralMMU Trainium2 kernel (v2: ACT-bound pipeline).

Per core: 131072 addrs, 64 iterations x 2048 addrs.

Engine plan per iteration t (steady state, ~1.9us period):
  ACT   Gelu(+b1): hpre slot(t) PSUM [128,2048] -> h(t) SBUF f32.
        One op per iter; this is the bottleneck engine (~1892 ns).
  PE    L2(t-1): 16 matmuls with SWAPPED operands: stationary lhsT =
        h(t-1)[:, 128c:128c+128] (f32, exact), moving rhs = W2 [128,26]
        f32 -> batch-major logits [128 batch, 26] written into the TAIL
        416 f32 of psum slot(t-1) (bank 3), which gelu(t-1) has already
        consumed.  26 cols * 4 cyc/row * 16 = 1664 cyc.
        L1(t+1): 4 bf16 matmuls k=96 (3-way bf16 split of W1, exact to
        ~2^-27) from host-prepared bf16 bit planes -> slot(t+1).
        Blocks g=0..2 issue early; block g=3 (tail bank) waits until the
        DVE threshold has read slot(t+1)'s previous logits.
  DVE   TT is_gt vs per-logit threshold vector (0.5 - b2[j], f32,
        partition-broadcast) -> bits bf16; TT mult by 2^(j%13) weight
        vector; tensor_reduce sum [128,16,2,13] -> packed lo/hi
        [128,32] f32 into an 8-iter accumulator.
  DMA   in: [96,4096] bf16 planes per 2 iters; out: [128,256] f32 per
        8 iters.  Host packs bit planes and combines lo+8192*hi.

PSUM: exactly 8 banks = 2 slots x [128,2048] f32; L2 output aliases the
tail of the slot (time-multiplexed with hpre data).

Numerics are f32-exact end-to-end except the 3-way-bf16 W1 split
(~2^-27) and the ACT Gelu LUT, identical to the f32 baseline (1/1M
mismatch there).
"""

import numpy as np
from contextlib import ExitStack

import concourse.bass as bass
import concourse.mybir as mybir
import concourse.tile as tile
from concourse import bacc, bass_utils

B = 1_048_576
NCORES = 8
PER = B // NCORES          # 131072 addrs per core
BLK = 512                  # addrs per L1 PE block
NBLK = 4                   # L1 blocks per iteration
CH = 128                   # addrs per L2 chunk (stationary width)
NCH = 16                   # L2 chunks per iteration
CHUNK = NBLK * BLK         # 2048 addrs per iteration
N_ITERS = PER // CHUNK     # 64
GIN = 2                    # iters per input DMA
GOUT = 8                   # iters per output DMA
NLOG = 26                  # logits per addr
LW = NCH * NLOG            # 416 logit cols per iteration
HLW = LW // 2              # 208: one thresh half

F32 = mybir.dt.float32
BF16 = mybir.dt.bfloat16
AF = mybir.ActivationFunctionType
ALU = mybir.AluOpType

# cst columns (f32): w1b 0:64 (bf16x128), b1c 64:65, w2f 65:91,
# wvec 91:299 (bf16 x416), thvec 299:715
CW1, CB1, CW2, CWV, CTH, CTOT = 0, 64, 65, 91, 299, 715


def build_nc(n_iters: int = N_ITERS, act=AF.Gelu) -> bass.Bass:
    nc = bacc.Bacc("TRN2")
    assert n_iters % GOUT == 0 and n_iters % GIN == 0

    bp = nc.dram_tensor("bp", [n_iters // GIN, 96, GIN * CHUNK], BF16,
                        kind="ExternalInput")
    cst_d = nc.dram_tensor("cst", [128, CTOT], F32, kind="ExternalInput")
    outp = nc.dram_tensor("outp", [n_iters // GOUT, 128, GOUT * 32], F32,
                          kind="ExternalOutput")

    with ExitStack() as ctx:
        tc = ctx.enter_context(tile.TileContext(nc))
        const = ctx.enter_context(tc.tile_pool(name="const", bufs=1))
        rpool = ctx.enter_context(tc.tile_pool(name="rp", bufs=3))
        ppool = ctx.enter_context(
            tc.tile_pool(name="ppool", bufs=1, space="PSUM"))
        hp = ctx.enter_context(tc.tile_pool(name="hp", bufs=2))
        bop = ctx.enter_context(tc.tile_pool(name="bop", bufs=2))
        bwp = ctx.enter_context(tc.tile_pool(name="bwp", bufs=2))
        pksp = ctx.enter_context(tc.tile_pool(name="pksp", bufs=2))

        # One persistent 8-bank psum tensor; all deps are subtile
        # (range-based), avoiding tile-granular pool-rotation WAR chains.
        PP = ppool.tile([128, 2 * CHUNK], F32, name="PP")

        cst = const.tile([128, CTOT], F32)
        nc.sync.dma_start(cst[:], cst_d[:])
        w1b = cst[:, CW1:CB1].bitcast(BF16)      # [128,128] bf16; rows 0-95
        b1c = cst[:, CB1:CW2]
        w2f = cst[:, CW2:CWV]                    # [128, 26] f32
        wv = cst[:, CWV:CTH].bitcast(BF16)       # [128, 416] bf16
        thv = cst[:, CTH:CTOT]                   # [128, 416] f32

        R = {}
        hs = {}
        bos = {}
        pks = None

        def half(t):
            return CHUNK * (t % 2)

        def load_input(g):
            if g < n_iters // GIN and g not in R:
                r = rpool.tile([96, GIN * CHUNK], BF16, name="r")
                nc.sync.dma_start(r[:], bp[g])
                R[g] = r

        def l1(t, blocks):
            """blocks: iterable of block indices.  Block 0's cols contain
            the previous era's logits, so it is issued last (after the
            DVE threshold has read them)."""
            if t >= n_iters:
                return
            r = R[t // GIN]
            for g in blocks:
                c0 = CHUNK * (t % GIN) + BLK * g
                nc.tensor.matmul(
                    PP[:, half(t) + BLK * g:half(t) + BLK * (g + 1)],
                    w1b[0:96, :],
                    r[0:96, c0:c0 + BLK],
                    start=True, stop=True, tile_position=(0, 0),
                )

        def gelu(t):
            h = hp.tile([128, CHUNK], F32, name="h")
            nc.scalar.activation(h[:], PP[:, half(t):half(t) + CHUNK], act,
                                 bias=b1c, scale=1.0)
            hs[t] = h

        def l2(t):
            h = hs.pop(t)
            for c in range(NCH):
                nc.tensor.matmul(
                    PP[:, half(t) + NLOG * c:half(t) + NLOG * (c + 1)],
                    h[:, CH * c:CH * (c + 1)],
                    w2f[:],
                    start=True, stop=True, tile_position=(0, 0),
                )

        def thresh(t, hi):
            """hi=0: logit cols 0:208 (chunks 0-7); hi=1: 208:416."""
            if not hi:
                bos[t] = bop.tile([128, LW], BF16, name="bo")
            o = HLW * hi
            nc.vector.tensor_tensor(
                bos[t][:, o:o + HLW],
                PP[:, half(t) + o:half(t) + o + HLW],
                thv[:, o:o + HLW],
                op=ALU.is_gt)

        def pack(t):
            nonlocal pks
            bw = bwp.tile([128, LW], BF16, name="bw")
            nc.vector.tensor_tensor(bw[:], bos.pop(t)[:], wv, op=ALU.mult)
            if t % GOUT == 0:
                pks = pksp.tile([128, GOUT * 32], F32, name="pks")
            nc.vector.tensor_reduce(
                pks[:, 32 * (t % GOUT):32 * (t % GOUT + 1)],
                bw[:].rearrange("p (g x) -> p g x", x=13),
                axis=mybir.AxisListType.X,
                op=ALU.add,
            )
            if t % GOUT == GOUT - 1:
                nc.sync.dma_start(outp[t // GOUT], pks[:])

        # Prologue: planes for iters 0-3, L1(0).
        load_input(0)
        load_input(1)
        l1(0, (1, 2, 3, 0))

        for t in range(n_iters):
            gelu(t)
            if t >= 1:
                l2(t - 1)
                thresh(t - 1, 0)
                thresh(t - 1, 1)
            if t % GIN == 0:
                load_input(t // GIN + 2)
            l1(t + 1, (1, 2, 3))
            if t >= 1:
                pack(t - 1)
            l1(t + 1, (0,))

        l2(n_iters - 1)
        thresh(n_iters - 1, 0)
        thresh(n_iters - 1, 1)
        pack(n_iters - 1)

    return nc


def make_const_inputs(W1, b1, W2, b2):
    import ml_dtypes

    w1 = np.ascontiguousarray(W1[0:32, :], dtype=np.float32)
    hi = w1.astype(ml_dtypes.bfloat16)
    mid = (w1 - hi.astype(np.float32)).astype(ml_dtypes.bfloat16)
    lo = (w1 - hi.astype(np.float32) - mid.astype(np.float32)).astype(
        ml_dtypes.bfloat16
    )
    w1b = np.zeros((128, 128), dtype=ml_dtypes.bfloat16)
    w1b[0:32] = hi
    w1b[32:64] = mid
    w1b[64:96] = lo

    cst = np.zeros((128, CTOT), dtype=np.float32)
    cst[:, CW1:CB1] = np.ascontiguousarray(w1b).view(np.float32)
    cst[:, CB1] = np.asarray(b1, dtype=np.float32)
    cst[:, CW2:CWV] = np.asarray(W2[:, :NLOG], dtype=np.float32)
    wvec = np.tile(
        np.concatenate([2.0 ** np.arange(13), 2.0 ** np.arange(13)]), NCH
    ).astype(ml_dtypes.bfloat16)        # [416]
    cst[:, CWV:CTH] = np.ascontiguousarray(wvec).view(np.float32)[None, :]
    thvec = np.tile(0.5 - np.asarray(b2[:NLOG], dtype=np.float32), NCH)
    cst[:, CTH:CTOT] = thvec[None, :]
    return {"cst": cst}


def make_bit_planes(virtual_addr, n_iters: int = N_ITERS):
    """Per-core [n_iters//GIN, 96, GIN*2048] bf16 0/1 bit planes.

    Partition 32s + k (s = 0..2 replication) of DMA group tt, col
    j*2048 + n = bit k of addr (GIN*tt + j)*2048 + n.
    """
    import ml_dtypes

    va32 = np.asarray(virtual_addr).astype(np.uint32)
    per = n_iters * CHUNK
    ncores = va32.size // per
    out = []
    for c in range(ncores):
        seg = va32[c * per:(c + 1) * per]
        byt = seg.view(np.uint8).reshape(n_iters // GIN, GIN * CHUNK, 4)
        bits = np.unpackbits(byt, axis=-1, bitorder="little")
        # (tt, n, k) -> (tt, k, n)
        pl = bits.transpose(0, 2, 1)
        pl3 = np.concatenate([pl, pl, pl], axis=1).astype(ml_dtypes.bfloat16)
        out.append(np.ascontiguousarray(pl3))
    return out


def combine_output(o, n_iters: int = N_ITERS):
    """[n_iters//GOUT, 128, GOUT*32] f32 -> [per] int64.

    col 32*ts + 2*c + half: lo/hi 13-bit halves of chunk c, iter
    GOUT*tt + ts; addr = CHUNK*t + CH*c + p.
    """
    arr = np.asarray(o, dtype=np.int64).reshape(
        n_iters // GOUT, 128, GOUT, NCH, 2)
    lo = arr[..., 0]                     # [tt, p, ts, c]
    hi = arr[..., 1]
    val = lo + 8192 * hi                 # [tt, p, ts, c]
    return val.transpose(0, 2, 3, 1).reshape(-1)


_NC_CACHE = {}
TRACE = False
LAST_RES = None


def kernel(virtual_addr, W1, b1, W2, b2):
    global LAST_RES
    if "nc" not in _NC_CACHE:
        nc = build_nc(N_ITERS)
        nc.finalize()
        _NC_CACHE["nc"] = nc
    nc = _NC_CACHE["nc"]

    consts = make_const_inputs(W1, b1, W2, b2)
    planes = make_bit_planes(virtual_addr, N_ITERS)
    in_maps = [{"bp": planes[c], **consts} for c in range(NCORES)]

    res = bass_utils.run_bass_kernel_spmd(
        nc, in_maps, list(range(NCORES)), trace=TRACE
    )
    LAST_RES = res

    outs = [combine_output(res.results[c]["outp"]) for c in range(NCORES)]
    return np.concatenate(outs)
